# revision 20
# baseline (speedup 1.0000x reference)
"""Causal self-attention for Trainium2, 8 NeuronCores.

Problem: B=2, T=2048, C=1024, H=16 heads (HD=64), fp32 reference.
Sharding: core = (batch b, head-group hg): b = core//4, hg = core%4; each core
computes 4 heads of one batch, producing ONE partial [T, C] projection output
in bf16 (both head-pairs accumulated in PSUM); the host sums the 4 partials
per batch.

Per-core compute (all matmuls bf16 -> fp32 psum):
  QKV:   qT/kT packs [128(=2 heads x 64), T] = w_pack.T @ x.T
         v packs [T, 128(=2 heads x 64)]
  Attn (transposed-S layout; head-pairs ALTERNATE per causal group so one
        pair's normalize chain hides under the other pair's j-loop):
         sAB[kj, qi] = kT.T @ qT   (both heads -> one 2-bank psum tile)
         eAB = exp(sAB/8)          single fused Act instruction per j
         causal via block slicing + triangle zero (Pool affine_select)
         psn[d|ones, qi] += v_tile.T @ eA  per head (denominator = row 64)
         yT = psn * (1/denom broadcast)    (copies + mul on DVE)
  Proj:  out[t, :] = sum over BOTH pairs of yTn_pack.T @ w_proj_pack,
         accumulated in psum, staged to bf16 SBUF by the Pool engine.
"""
import sys

if "/opt/trn_rl_repo" not in sys.path:
    sys.path.insert(0, "/opt/trn_rl_repo")

import numpy as np
import ml_dtypes

import concourse.bass as bass
import concourse.tile as tile
import concourse.mybir as mybir
from concourse.bass_utils import run_bass_kernel_spmd

B, T, C, H, HD = 2, 2048, 1024, 16, 64
P = 128
CK = C // P          # 8 c-chunks
G = 4                # qi groups of 512
NG = T // G          # 512
KT = T // P          # 16 kj tiles
HPC = 4              # heads per core
N_CORES = 8
BF16 = mybir.dt.bfloat16
F32 = mybir.dt.float32
SCALE = 1.0 / 8.0    # 1/sqrt(HD)


def _split_excess_waits(nc):
    # walrus in this container accepts at most ONE semaphore wait per
    # instruction; move extras onto same-engine NOPs inserted just before.
    ctr = 0
    for fn in nc.m.functions:
        for bb in fn.blocks:
            out = []
            changed = False
            for inst in bb.instructions:
                si = inst.sync_info
                waits = list(si.on_wait) if si is not None and si.on_wait else []
                if len(waits) > 1:
                    for w in waits[:-1]:
                        nop = mybir.InstNoOp(
                            name=f"waitsplit-{ctr}",
                            engine=inst.engine,
                            ins=[],
                            outs=[],
                            sync_info=mybir.SyncInfo(on_wait=[w], on_update=[]),
                        )
                        ctr += 1
                        out.append(nop)
                    si.on_wait = waits[-1:]
                    changed = True
                out.append(inst)
            if changed:
                bb.instructions[:] = out
    return ctr


def build(debug=False):
    nc = bass.Bass(trn_type="TRN2")
    xT = nc.dram_tensor("xT", (C, T), BF16, kind="ExternalInput")
    wq = nc.dram_tensor("wq", (2, C, P), BF16, kind="ExternalInput")
    wk = nc.dram_tensor("wk", (2, C, P), BF16, kind="ExternalInput")
    wv = nc.dram_tensor("wv", (C, 2 * P), BF16, kind="ExternalInput")
    wp = nc.dram_tensor("wp", (2 * P, C), BF16, kind="ExternalInput")
    out = nc.dram_tensor("out", (T, C), BF16, kind="ExternalOutput")

    with tile.TileContext(nc) as tc:
        with (
            tc.tile_pool(name="const", bufs=1) as const,
            tc.tile_pool(name="big", bufs=1) as big,
            tc.tile_pool(name="expp", bufs=4) as expp,
            tc.tile_pool(name="stage", bufs=3) as stage,
            tc.tile_pool(name="bcp", bufs=2) as bcp,
        ):
            # ---- persistent SBUF tensors ----
            xT_sb = const.tile([P, CK, T], BF16)
            wq_sb = const.tile([P, 2, CK, P], BF16)
            wk_sb = const.tile([P, 2, CK, P], BF16)
            wv_sb = const.tile([P, CK, 2 * P], BF16)
            wp_sb = const.tile([P, 2, C], BF16)
            qT = [big.tile([P, T], BF16, tag=f"qT{p}", name=f"qT{p}") for p in range(2)]
            kT = [big.tile([P, T], BF16, tag=f"kT{p}", name=f"kT{p}") for p in range(2)]
            vp = [big.tile([P, KT, 2, 65], BF16, tag=f"vp{p}", name=f"vp{p}") for p in range(2)]
            yT = [big.tile([P, T], BF16, tag=f"yT{p}", name=f"yT{p}") for p in range(2)]

            for p_ in range(2):
                nc.vector.memset(vp[p_][:, :, :, 64:65], 1.0)

            ones64 = const.tile([1, 64], BF16)
            nc.vector.memset(ones64, 1.0)

            # ---- input DMAs, spread across the two HWDGE queues (SP, Act)
            # so xT chunks and weights land in parallel ----
            wq_r = wq.rearrange("pk (ko p) m -> p pk ko m", p=P)
            wk_r = wk.rearrange("pk (ko p) m -> p pk ko m", p=P)
            xT_r = xT.rearrange("(ko p) t -> p ko t", p=P)
            for pk in range(2):
                nc.sync.dma_start(wq_sb[:, pk], wq_r[:, pk])
            nc.scalar.dma_start(xT_sb[:, 0], xT_r[:, 0])
            nc.scalar.dma_start(xT_sb[:, 1], xT_r[:, 1])
            for ko in range(2, CK):
                eng = nc.sync if ko % 2 == 0 else nc.scalar
                eng.dma_start(xT_sb[:, ko], xT_r[:, ko])
            for pk in range(2):
                nc.scalar.dma_start(wk_sb[:, pk], wk_r[:, pk])
            nc.sync.dma_start(wv_sb, wv.rearrange("(ko p) m -> p ko m", p=P))
            nc.sync.dma_start(wp_sb, wp.rearrange("(po p) n -> p po n", p=P))
            # warm the Exp activation table while the input DMAs stream in,
            # so the first real exp doesn't pay the ~1.3us table load
            scratch = const.tile([1, 64], BF16)
            nc.scalar.activation(scratch, ones64,
                                 mybir.ActivationFunctionType.Exp, scale=SCALE)

            # ================= QKV =================
            with tc.tile_pool(name="psqkv", bufs=4, space="PSUM") as psq:
                # Q: ko-outer so compute starts after the first xT chunk lands
                for pk in range(2):
                    pss = [psq.tile([P, NG], F32, tag="psq", name=f"psq_{pk}_{i}") for i in range(G)]
                    for ko in range(CK):
                        for t4 in range(G):
                            nc.tensor.matmul(
                                pss[t4],
                                wq_sb[:, pk, ko],
                                xT_sb[:, ko, t4 * NG:(t4 + 1) * NG],
                                start=(ko == 0),
                                stop=(ko == CK - 1),
                            )
                    for t4 in range(G):
                        nc.vector.tensor_copy(qT[pk][:, t4 * NG:(t4 + 1) * NG], pss[t4])
                # K
                for pk in range(2):
                    for t4 in range(G):
                        ps = psq.tile([P, NG], F32, tag="psq")
                        for ko in range(CK):
                            nc.tensor.matmul(
                                ps,
                                wk_sb[:, pk, ko],
                                xT_sb[:, ko, t4 * NG:(t4 + 1) * NG],
                                start=(ko == 0),
                                stop=(ko == CK - 1),
                            )
                        nc.vector.tensor_copy(kT[pk][:, t4 * NG:(t4 + 1) * NG], ps)
                # V: out [t 128, 256]; cols 0:128 pair0, 128:256 pair1
                for t in range(KT):
                    ps = psq.tile([P, NG], F32, tag="psq")
                    for ko in range(CK):
                        nc.tensor.matmul(
                            ps[:, :2 * P],
                            xT_sb[:, ko, t * P:(t + 1) * P],
                            wv_sb[:, ko],
                            start=(ko == 0),
                            stop=(ko == CK - 1),
                        )
                    for p_ in range(2):
                        nc.vector.tensor_copy(
                            vp[p_][:, t, :, 0:64],
                            ps[:, 128 * p_:128 * (p_ + 1)].rearrange(
                                "p (h d) -> p h d", h=2))

            # ================= Attention + Proj =================
            # PSUM: 4 numerator banks (2 per pair) + one shared 2-buffer pool
            # of [128,1024] tiles (tag "sab", 4 banks) used by BOTH the S
            # matmuls and the projection accumulators.
            with tc.tile_pool(name="psnum", bufs=1, space="PSUM") as psn_pool, \
                 tc.tile_pool(name="psab", bufs=2, space="PSUM") as psab:

                def blk(g, j):
                    r = j - 4 * g
                    c0 = max(r, 0) * P
                    return r, c0, NG - c0, NG * g + c0

                def emit_s(pair, g, j):
                    _, c0, width, qi0 = blk(g, j)
                    sAB = psab.tile([P, 2 * NG], F32, tag="sab",
                                    name=f"sAB{pair}_{g}_{j}")
                    nc.tensor.matmul(
                        sAB[:, 0:width],
                        kT[pair][0:64, j * P:(j + 1) * P],
                        qT[pair][0:64, qi0:qi0 + width],
                        start=True, stop=True,
                        tile_position=(0, 0),
                    )
                    nc.tensor.matmul(
                        sAB[:, NG:NG + width],
                        kT[pair][64:128, j * P:(j + 1) * P],
                        qT[pair][64:128, qi0:qi0 + width],
                        start=True, stop=True,
                        tile_position=(64, 0),
                    )
                    return sAB

                # flat attention stream: groups in causal order, pairs
                # interleaved at the j level (consecutive items independent,
                # so one item's exp hides under the other pair's matmuls)
                attn_items = []
                for g in range(G):
                    for pair in range(2):
                        njs = 4 * g + 4
                        for j in range(njs):
                            attn_items.append((pair, g, j, njs))

                # numerator psum tiles, created lazily per (pair, g)
                cur_psn = {}

                def numer_tiles(pair, g):
                    nA = psn_pool.tile([P, NG], F32, tag=f"nA{pair}",
                                       name=f"nA{pair}_{g}")
                    nB = psn_pool.tile([P, NG], F32, tag=f"nB{pair}",
                                       name=f"nB{pair}_{g}")
                    return nA, nB

                def normalize_a(pair, g):
                    # group end: extract numerator + denominators (DVE only,
                    # no PE instructions -> doesn't stall the PE queue)
                    nA, nB = cur_psn[(pair, g)]
                    gs = slice(NG * g, NG * (g + 1))
                    nc.vector.tensor_copy(yT[pair][0:64, gs], nA[0:64])
                    nc.vector.tensor_copy(yT[pair][64:128, gs], nB[0:64])
                    dgA = bcp.tile([1, NG], BF16, tag="dgA", name=f"dgA{pair}_{g}")
                    dgB = bcp.tile([1, NG], BF16, tag="dgB", name=f"dgB{pair}_{g}")
                    with nc.allow_low_precision(reason="1/denom in bf16 is plenty"):
                        nc.vector.reciprocal(dgA, nA[64:65, :])
                        nc.vector.reciprocal(dgB, nB[64:65, :])
                    cur_dg[(pair, g)] = (dgA, dgB)

                def normalize_b(pair, g):
                    # deferred: broadcast 1/denom across 64 partitions via a
                    # K=1 matmul into the (already-extracted) numerator banks,
                    # then scale yT in place
                    nA, nB = cur_psn.pop((pair, g))
                    dgA, dgB = cur_dg.pop((pair, g))
                    gs = slice(NG * g, NG * (g + 1))
                    nc.tensor.matmul(nA[0:64, :], ones64, dgA, start=True, stop=True)
                    nc.tensor.matmul(nB[0:64, :], ones64, dgB, start=True, stop=True)
                    nc.vector.tensor_mul(yT[pair][0:64, gs], yT[pair][0:64, gs],
                                         nA[0:64])
                    nc.vector.tensor_mul(yT[pair][64:128, gs], yT[pair][64:128, gs],
                                         nB[0:64])

                def proj_block(g, t):
                    # one 128-token block; contracts all 4 heads (both pairs)
                    ts = slice(t * P, (t + 1) * P)
                    pj = psab.tile([P, 2 * NG], F32, tag="sab", name=f"pj{t}")
                    for half in range(2):
                        cs = slice(half * NG, (half + 1) * NG)
                        for pk in range(2):
                            nc.tensor.matmul(
                                pj[:, cs],
                                yT[pk][:, ts],
                                wp_sb[:, pk, cs],
                                start=(pk == 0),
                                stop=(pk == 1),
                            )
                    # GPSIMD cannot read PSUM on real HW -> stage on DVE
                    st = stage.tile([P, 2 * NG], BF16, tag="st", name=f"st{t}")
                    nc.vector.tensor_copy(st, pj)
                    eng = nc.scalar if (g == G - 1 and t % 2 == 1) else nc.sync
                    eng.dma_start(out[ts, :], st)

                # deferred work keyed by the attn-item index at which it may
                # be emitted: normalize_b two items after its group ends,
                # proj blocks spliced one-per-item a few items later still
                cur_dg = {}
                deferred = {}

                def defer(idx, fn):
                    deferred.setdefault(idx, []).append(fn)

                for i, it in enumerate(attn_items):
                    pair, g, j, njs = it
                    if j == njs - 1:
                        defer(min(i + 2, len(attn_items) - 1),
                              (lambda p_, g_: lambda: normalize_b(p_, g_))(pair, g))
                        if pair == 1:
                            for ti, t in enumerate(range(4 * g, 4 * g + 4)):
                                defer(min(i + 3 + ti, len(attn_items) - 1),
                                      (lambda g_, t_: lambda: proj_block(g_, t_))(g, t))

                # 1-deep S prefetch over the attn stream
                s_next = emit_s(*attn_items[0][:3])

                for i, tk in enumerate(attn_items):
                    pair, g, j, njs = tk
                    r, c0, width, qi0 = blk(g, j)
                    sAB = s_next
                    if i + 1 < len(attn_items):
                        s_next = emit_s(*attn_items[i + 1][:3])
                    eAB = expp.tile([P, 2, NG], BF16, tag="eAB")
                    # single fused exp over both heads' S blocks
                    sview = bass.AP(
                        tensor=sAB.tensor, offset=sAB.offset,
                        ap=[list(sAB.ap[0]), [NG, 2], [1, width]],
                    )
                    eview = bass.AP(
                        tensor=eAB.tensor, offset=eAB.offset,
                        ap=[list(eAB.ap[0]), [NG, 2], [1, width]],
                    )
                    nc.scalar.activation(
                        eview, sview,
                        mybir.ActivationFunctionType.Exp, scale=SCALE,
                    )
                    if r >= 0:
                        # zero the strictly-lower triangle (kj > qi)
                        for h in range(2):
                            nc.gpsimd.affine_select(
                                out=eAB[:, h, 0:P], in_=eAB[:, h, 0:P],
                                compare_op=mybir.AluOpType.is_ge,
                                fill=0.0, base=0,
                                pattern=[[1, P]], channel_multiplier=-1,
                            )
                    if j == 0:
                        cur_psn[(pair, g)] = numer_tiles(pair, g)
                    nA, nB = cur_psn[(pair, g)]
                    last = j == njs - 1
                    nc.tensor.matmul(
                        nA[0:65, c0:NG], vp[pair][:, j, 0],
                        eAB[:, 0, :width], start=(j == 0), stop=last,
                    )
                    nc.tensor.matmul(
                        nB[0:65, c0:NG], vp[pair][:, j, 1],
                        eAB[:, 1, :width], start=(j == 0), stop=last,
                    )
                    if last:
                        normalize_a(pair, g)
                    for fn in deferred.pop(i, []):
                        fn()

    _split_excess_waits(nc)
    return nc


_NC = None


def kernel(x, w_attn, b_attn, w_proj, b_proj):
    global _NC
    if _NC is None:
        _NC = build()
    bf = ml_dtypes.bfloat16

    xT = [np.ascontiguousarray(x[b].T).astype(bf) for b in range(B)]
    in_maps = []
    for core in range(N_CORES):
        b, hg = divmod(core, HPC)
        h0 = hg * HPC  # first head of this core
        c0 = h0 * HD   # first column within each of q/k/v blocks
        wq_l = w_attn[:, c0:c0 + HPC * HD]
        wk_l = w_attn[:, C + c0:C + c0 + HPC * HD]
        wv_l = w_attn[:, 2 * C + c0:2 * C + c0 + HPC * HD]
        wp_l = w_proj[c0:c0 + HPC * HD, :]
        in_maps.append({
            "xT": xT[b],
            "wq": np.ascontiguousarray(
                wq_l.reshape(C, 2, 2 * HD).transpose(1, 0, 2)).astype(bf),
            "wk": np.ascontiguousarray(
                wk_l.reshape(C, 2, 2 * HD).transpose(1, 0, 2)).astype(bf),
            "wv": np.ascontiguousarray(wv_l).astype(bf),
            "wp": np.ascontiguousarray(wp_l).astype(bf),
        })

    res = run_bass_kernel_spmd(_NC, in_maps, core_ids=list(range(N_CORES)))
    out = np.zeros((B, T, C), dtype=np.float32)
    for core in range(N_CORES):
        b = core // HPC
        out[b] += res.results[core]["out"].astype(np.float32)
    out += np.asarray(b_proj, dtype=np.float32)
    return out
